# revision 24
# baseline (speedup 1.0000x reference)
"""ECC (edge-conditioned convolution) GNN message passing on 8 NeuronCores.

Strategy
--------
Edges are sorted by destination node (host side) and split into 8
contiguous, segment-aligned shards -- one per core.  Each core runs an
identical Bass program over its shard, tiled 512 edges at a time:

  PE  : h1 = relu(W1 @ eaT); h2 = relu(W2 @ h1);
        theta blocks (8x [128,512] per tile) = w3T_block.T @ h2;
        8 col-tiled selector matmuls (M=8, tile_position=(0,32j)) reduce
        prod over i -- four 32-column PE groups run concurrently, so the
        selector pass costs ~2 matmul spans instead of 8.
  ACT : PSUM->SBUF evacuation (relu for h1/h2, f32->bf16 casts for theta)
  DVE : theta * xs_rep elementwise (bf16 2x mode; the deferred "A"-pair
        multiplies of a tile are fused into ONE wide op), one fused
        evac-multiply straight from PSUM (scalar_tensor_tensor), and a
        masked prefix scan (tensor_tensor_scan) with initial=0 PER TILE --
        the host stitches segments that span tile boundaries by adding the
        earlier tiles' last scan columns, which keeps the scans cheap and
        independent (no serial cross-tile carry).
  Pool: one pair of SBUF-only multiplies to offload DVE.

msg rows live scattered across PSUM partitions (p = 32*colgrp + m); the
host knows the permutation.  The scan output [128, e_c] bf16 goes back to
HBM; the host reads each segment's last column (plus boundary partials),
divides by degree and applies the final relu.  Shards are segment aligned
so no cross-core reduction is needed.
"""

import math
import sys

import numpy as np

for _p in ("/opt/trn_rl_repo", "/root/.axon_site/_ro/trn_rl_repo"):
    if _p not in sys.path:
        sys.path.insert(0, _p)

import ml_dtypes

import concourse.bass as bass
import concourse.mybir as mybir
import concourse.tile as tile
from concourse import bacc
from concourse.bass_utils import run_bass_kernel_spmd

N_NODES = 25000
N_EDGES = 250000
F_IN = 32
F_OUT = 32
EDGE_DIM = 6
H1, H2 = 64, 128
N_CORES = 8
E_TILE = 512

BF16 = ml_dtypes.bfloat16

_program_cache: dict = {}

# per-pair conveyor modes for even/odd tiles:
#   "A" = ACT evac + DVE mult, "P" = ACT evac + Pool mult, "S" = DVE fused STT
DEFAULT_CFG = {
    "modes_even": ("A", "A", "P", "S"),
    "modes_odd": ("A", "A", "P", "S"),
    "h1s_even": "act",
    "h1s_odd": "act",
    "h2s_even": "act",
    "h2s_odd": "act",
    # pair processing order on PE (STT pairs first so DVE can drain them
    # while ACT works through the evac queue)
    "pair_order": (1, 3, 2, 0),
    # tiles of lag before a tile's selectors+scan run (deeper = more slack
    # for the slow Pool pair's products)
    "back_delay": 3,
    # merge each tile's deferred A-pair mults into one wide DVE op
    "fuse_amuls": True,
    # one scan op per tile pair (PSUM msg tile spans the pair)
    "fuse_scan": False,
    # 8-row rank-1 mask + 32-row output DMAs (saves ~12MB HBM per core);
    # measured slightly slower (SP op count) -- keep off
    "slim_dma": False,
}


def _sel_matrices():
    sel = np.zeros((128, 16), dtype=np.float32)
    rows = np.arange(128)
    for m in range(4):
        sel[rows[rows // 32 == m], m] = 1.0          # sel_lo: cols 0..3
        sel[rows[rows // 32 == m], 8 + 4 + m] = 1.0  # sel_hi: cols 12..15
    return sel.astype(BF16)


def _perm():
    """P[o] = psum partition holding msg row o."""
    P = np.empty(F_OUT, dtype=np.int64)
    for o in range(F_OUT):
        if o < 16:
            P[o] = 32 * (o // 4) + o % 4
        else:
            P[o] = 32 * (o // 4 - 4) + 4 + o % 4
    return P


def _build_program(
    e_c: int,
    bench_repeat: int | None = None,
    has_b3: bool = False,
    cfg: dict | None = None,
) -> "bass.Bass":
    cfg = {**DEFAULT_CFG, **(cfg or {})}
    f32 = mybir.dt.float32
    bf16 = mybir.dt.bfloat16
    n_tiles = e_c // E_TILE
    assert e_c % (2 * E_TILE) == 0

    nc = bacc.Bacc(None, target_bir_lowering=False)
    slim = bool(cfg.get("slim_dma"))

    ea_t_d = nc.declare_dram_parameter("eaT", [EDGE_DIM, e_c], bf16, isOutput=False)
    xs_d = nc.declare_dram_parameter("xsrep", [128, e_c], bf16, isOutput=False)
    mk_d = nc.declare_dram_parameter("mask", [8 if slim else 128, e_c], bf16, isOutput=False)
    w1_d = nc.declare_dram_parameter("w1T", [EDGE_DIM, H1], bf16, isOutput=False)
    w2_d = nc.declare_dram_parameter("w2T", [H1, H2], bf16, isOutput=False)
    w3_d = nc.declare_dram_parameter("w3T", [H2, F_OUT * F_IN], bf16, isOutput=False)
    sel_d = nc.declare_dram_parameter("sel", [128, 16], bf16, isOutput=False)
    b1_d = nc.declare_dram_parameter("b1v", [H1, 1], f32, isOutput=False)
    b2_d = nc.declare_dram_parameter("b2v", [H2, 1], f32, isOutput=False)
    if has_b3:
        b3_d = nc.declare_dram_parameter("b3m", [128, 8], f32, isOutput=False)
    out_d = nc.declare_dram_parameter("scan_out", [32 if slim else 128, e_c], bf16, isOutput=True)

    relu = mybir.ActivationFunctionType.Relu
    copy = mybir.ActivationFunctionType.Copy
    mult = mybir.AluOpType.mult
    add = mybir.AluOpType.add

    with tile.TileContext(nc) as tc:
        with (
            tc.tile_pool(name="const", bufs=1) as const,
            tc.tile_pool(name="io", bufs=3) as io,
            tc.tile_pool(name="mid", bufs=3) as mid,
            tc.tile_pool(name="scanb", bufs=2) as scanb,
            tc.tile_pool(name="psA", bufs=1, space="PSUM") as psA,
            tc.tile_pool(name="psTH", bufs=2, space="PSUM") as psTH,
            tc.tile_pool(name="psB", bufs=2, space="PSUM") as psB,
        ):
            s_w1 = const.tile([EDGE_DIM, H1], bf16)
            nc.sync.dma_start(out=s_w1, in_=w1_d[:])
            s_w2 = const.tile([H1, H2], bf16)
            nc.sync.dma_start(out=s_w2, in_=w2_d[:])
            s_w3 = const.tile([H2, F_OUT * F_IN], bf16)
            nc.sync.dma_start(out=s_w3, in_=w3_d[:])
            s_sel = const.tile([128, 16], bf16)
            nc.sync.dma_start(out=s_sel, in_=sel_d[:])
            s_b1 = const.tile([H1, 1], f32)
            nc.sync.dma_start(out=s_b1, in_=b1_d[:])
            s_b2 = const.tile([H2, 1], f32)
            nc.sync.dma_start(out=s_b2, in_=b2_d[:])
            if has_b3:
                s_b3 = const.tile([128, 8], f32)
                nc.sync.dma_start(out=s_b3, in_=b3_d[:])

            import contextlib

            loop_cm = (
                tc.For_i(
                    0,
                    bench_repeat,
                    1,
                    hint_engines=(
                        mybir.EngineType.PE,
                        mybir.EngineType.Activation,
                        mybir.EngineType.DVE,
                        mybir.EngineType.SP,
                        mybir.EngineType.Pool,
                    ),
                )
                if bench_repeat is not None
                else contextlib.nullcontext()
            )
            with loop_cm:
                state = {"sc2": None, "pending_out": [], "pending_mults": []}

                def emit_back(pend):
                    """Selectors + scan (+ out DMA) for a completed tile."""
                    t = pend["t"]
                    par = t % 2
                    if cfg.get("fuse_scan"):
                        # selectors of a tile pair share one 2-bank PSUM tile
                        # so the pair needs only ONE scan op
                        if par == 0:
                            state["msgp2"] = psB.tile(
                                [128, 2, E_TILE], f32, tag="msg2", bufs=1, name="msgp2"
                            )
                        msgp = state["msgp2"][:, par, :]
                    else:
                        msgp = psB.tile(
                            [128, E_TILE], f32, tag="msg", name="msgp",
                            bufs=1 if cfg.get("h2_pp") else 2,
                        )
                    for p in (() if cfg.get("ablate_sel") else cfg.get("sel_order", (0, 1, 3, 2))):
                        prod2 = pend["prods"][p]
                        for h in range(2):
                            b = 2 * p + h
                            j = b % 4
                            sel_cols = s_sel[:, 0:8] if b < 4 else s_sel[:, 8:16]
                            nc.tensor.matmul(
                                msgp[32 * j : 32 * j + 8, :],
                                sel_cols,
                                prod2[:, h, :],
                                start=(b < 4),
                                stop=(b >= 4),
                                tile_position=(0, 32 * j),
                                skip_group_check=True,
                            )
                    if cfg.get("ablate_scan"):
                        return
                    if par == 0:
                        state["sc2"] = scanb.tile([128, 2, E_TILE], bf16, tag="sc", name="sc2")
                    sc2 = state["sc2"]
                    # per-tile(-pair) scan with initial=0: the host stitches
                    # segments that span scan-reset boundaries (adds the
                    # previous windows' last columns).  An AP initial would
                    # cost ~+534ns on DVE and serialize the scans.
                    sc_flat = bass.AP(
                        tensor=sc2.tensor,
                        offset=sc2.offset,
                        ap=[list(sc2.ap[0]), [1, 2 * E_TILE]],
                    )
                    if cfg.get("fuse_scan"):
                        if par == 1:
                            msg_flat = bass.AP(
                                tensor=state["msgp2"].tensor,
                                offset=state["msgp2"].offset,
                                ap=[list(state["msgp2"].ap[0]), [1, 2 * E_TILE]],
                            )
                            nc.vector.tensor_tensor_scan(
                                sc_flat, pend["mk2"], msg_flat, initial=0.0,
                                op0=mult, op1=add,
                            )
                    else:
                        sc_slice = sc2[:, par, :]
                        nc.vector.tensor_tensor_scan(
                            sc_slice, pend["mk_t"], msgp, initial=0.0, op0=mult, op1=add
                        )
                    if par == 1:
                        lo2 = (t - 1) * E_TILE
                        if slim:
                            # only the 32 rows holding real msg data (the P
                            # permutation: rows 32g..32g+7 for g in 0..3)
                            for g in range(4):
                                state["pending_out"].append(
                                    (
                                        out_d[8 * g : 8 * g + 8, lo2 : lo2 + 2 * E_TILE],
                                        sc_flat[32 * g : 32 * g + 8, :],
                                    )
                                )
                        else:
                            state["pending_out"].append(
                                (out_d[:, lo2 : lo2 + 2 * E_TILE], sc_flat)
                            )

                def flush_out():
                    # issue out-DMAs late so their waits are already met and
                    # never head-block the SP sequencer's input prefetches
                    for out_ap, in_ap in state["pending_out"]:
                        nc.sync.dma_start(out=out_ap, in_=in_ap)
                    state["pending_out"] = []

                xs2t = mk2t = ea2t = None
                pendq = []
                for t in range(n_tiles):
                    par = t % 2
                    modes = cfg["modes_even"] if par == 0 else cfg["modes_odd"]
                    h1s_eng = cfg["h1s_even"] if par == 0 else cfg["h1s_odd"]
                    h2s_eng = cfg["h2s_even"] if par == 0 else cfg["h2s_odd"]
                    lo2 = (t - par) * E_TILE
                    if par == 0:
                        xs2t = io.tile([128, 2 * E_TILE], bf16, tag="xs")
                        nc.sync.dma_start(out=xs2t, in_=xs_d[:, lo2 : lo2 + 2 * E_TILE])
                        mk2t = io.tile([128, 2 * E_TILE], bf16, tag="mk")
                        if slim:
                            # rank-1 mask: 8 identical host rows, broadcast to
                            # the 4 row-groups the scan's useful rows live in;
                            # the other 96 rows keep stale data (ignored)
                            for g in range(4):
                                nc.sync.dma_start(
                                    out=mk2t[32 * g : 32 * g + 8, :],
                                    in_=mk_d[:, lo2 : lo2 + 2 * E_TILE],
                                )
                        else:
                            nc.sync.dma_start(out=mk2t, in_=mk_d[:, lo2 : lo2 + 2 * E_TILE])
                        ea2t = io.tile([EDGE_DIM, 2 * E_TILE], bf16, tag="ea")
                        nc.sync.dma_start(
                            out=ea2t, in_=ea_t_d[:, lo2 : lo2 + 2 * E_TILE]
                        )
                        flush_out()

                    c0 = par * E_TILE
                    ea_t = ea2t[:, c0 : c0 + E_TILE]
                    xs_t = xs2t[:, c0 : c0 + E_TILE]
                    mk_t = mk2t[:, c0 : c0 + E_TILE]

                    with tc.high_priority(offset=cfg.get("hprio", 0) or None) if cfg.get("hprio") else contextlib.nullcontext():
                        h1p = psA.tile([H1, E_TILE], f32, tag="h1")
                        nc.tensor.matmul(h1p, s_w1, ea_t, start=True, stop=True)
                        h1s = mid.tile([H1, E_TILE], bf16, tag="h1s")
                        if h1s_eng == "act":
                            nc.scalar.activation(h1s, h1p, relu, bias=s_b1)
                        else:
                            nc.vector.tensor_scalar(
                                h1s, h1p, s_b1, 0.0, add, mybir.AluOpType.max
                            )

                        h2p = psA.tile(
                            [H2, E_TILE], f32, tag="h2",
                            bufs=2 if cfg.get("h2_pp") else 1,
                        )
                        nc.tensor.matmul(h2p, s_w2, h1s, start=True, stop=True)
                        h2s = mid.tile([H2, E_TILE], bf16, tag="h2s")
                        if h2s_eng == "act":
                            nc.scalar.activation(h2s, h2p, relu, bias=s_b2)
                        else:
                            nc.vector.tensor_scalar(
                                h2s, h2p, s_b2, 0.0, add, mybir.AluOpType.max
                            )

                    # lagged tiles' selectors+scan fill the PE while ACT
                    # works through h1s/h2s for this tile
                    if len(pendq) >= cfg["back_delay"]:
                        emit_back(pendq.pop(0))

                    # broadcast view of xs_t: [128, 2(bcast), 512]
                    xs2 = bass.AP(
                        tensor=xs_t.tensor,
                        offset=xs_t.offset,
                        ap=[list(xs_t.ap[0]), [0, 2], list(xs_t.ap[1])],
                    )
                    prods = [None] * 4
                    agrp = None
                    fuse = bool(cfg.get("fuse_amuls"))
                    n_A = sum(1 for q in modes if q == "A")
                    for p in cfg["pair_order"]:
                        thp2 = psTH.tile([128, 2, E_TILE], f32, tag="th")
                        for h in range(2):
                            b = 2 * p + h
                            nc.tensor.matmul(
                                thp2[:, h, :],
                                s_w3[:, b * 128 : (b + 1) * 128],
                                h2s,
                                start=True,
                                stop=True,
                            )
                        mode = modes[p]
                        prod2 = (
                            None
                            if mode == "A"
                            else mid.tile(
                                [128, 2, E_TILE], bf16, tag="prod", bufs=13,
                                name="prod2",
                            )
                        )
                        if mode == "S" and not has_b3:
                            nc.vector.scalar_tensor_tensor(
                                prod2, thp2, 1.0, xs2, mult, mult
                            )
                        elif mode == "S":  # has_b3: per-block, bias added
                            for h in range(2):
                                b = 2 * p + h
                                nc.vector.scalar_tensor_tensor(
                                    prod2[:, h, :],
                                    thp2[:, h, :],
                                    s_b3[:, b : b + 1],
                                    xs_t,
                                    add,
                                    mult,
                                )
                        else:
                            if mode == "A":
                                # defer the DVE mult one tile: keeps the DVE
                                # FIFO free of ACT-gated ops (prods are only
                                # consumed back_delay tiles later).  With
                                # fuse_amuls all A pairs of a tile share one
                                # group tile so the deferred mult is a single
                                # wide DVE op.
                                if agrp is None:
                                    w = 2 * n_A if fuse else 2
                                    agrp = {
                                        "ths": mid.tile(
                                            [128, w, E_TILE], bf16, tag="thsg",
                                            bufs=6 if fuse else 12, name="thsg",
                                        ),
                                        "w": w,
                                        "used": 0,
                                        "slots": [],
                                        "prods": prods,
                                        "xs_t": xs_t,
                                    }
                                    state["pending_mults"].append(agrp)
                                idx = agrp["used"]
                                agrp["used"] += 1
                                ths2 = agrp["ths"][:, 2 * idx : 2 * idx + 2, :]
                                agrp["slots"].append((p, idx))
                                if not fuse:
                                    agrp = None
                            else:
                                ths2 = mid.tile([128, 2, E_TILE], bf16, tag="ths", bufs=6)
                            if not has_b3:
                                nc.scalar.activation(ths2, thp2, copy)
                            else:
                                for h in range(2):
                                    b = 2 * p + h
                                    nc.scalar.activation(
                                        ths2[:, h, :],
                                        thp2[:, h, :],
                                        copy,
                                        bias=s_b3[:, b : b + 1],
                                    )
                            if mode == "A":
                                prod2 = None
                            else:  # "P": Pool, per-block plain APs
                                for h in range(2):
                                    nc.gpsimd.tensor_tensor(
                                        prod2[:, h, :], ths2[:, h, :], xs_t, mult
                                    )
                        prods[p] = prod2

                    # flush previous tile's deferred DVE mults (their evacs
                    # finished long ago, so they never stall the DVE FIFO)
                    def flush_mults(m):
                        w = m["w"]
                        mp = mid.tile(
                            [128, w, E_TILE], bf16, tag="prodg",
                            bufs=6 if fuse else 12, name="prodd",
                        )
                        xt = m["xs_t"]
                        xsb = bass.AP(
                            tensor=xt.tensor,
                            offset=xt.offset,
                            ap=[list(xt.ap[0]), [0, w], list(xt.ap[1])],
                        )
                        nc.vector.tensor_mul(mp, m["ths"], xsb)
                        for p_, idx in m["slots"]:
                            m["prods"][p_] = mp[:, 2 * idx : 2 * idx + 2, :]

                    newly = [m for m in state["pending_mults"] if m["prods"] is not prods]
                    for m in newly:
                        flush_mults(m)
                    state["pending_mults"] = [
                        m for m in state["pending_mults"] if m["prods"] is prods
                    ]

                    pendq.append({"t": t, "prods": prods, "mk_t": mk_t, "mk2": mk2t})
                for m in state["pending_mults"]:
                    flush_mults(m)
                state["pending_mults"] = []
                for pd in pendq:
                    emit_back(pd)
                flush_out()

    nc.finalize()
    return nc


def _prepare(x, edge_attr, W1, b1, W2, b2, W3, b3, edge_src, edge_dst, cfg=None):
    cfg = {**DEFAULT_CFG, **(cfg or {})}
    slim = bool(cfg.get("slim_dma"))
    x = np.asarray(x, dtype=np.float32)
    edge_attr = np.asarray(edge_attr, dtype=np.float32)
    W1 = np.asarray(W1, dtype=np.float32)
    b1 = np.asarray(b1, dtype=np.float32)
    W2 = np.asarray(W2, dtype=np.float32)
    b2 = np.asarray(b2, dtype=np.float32)
    W3 = np.asarray(W3, dtype=np.float32)
    b3 = np.asarray(b3, dtype=np.float32)
    edge_src = np.asarray(edge_src).astype(np.int64)
    edge_dst = np.asarray(edge_dst).astype(np.int64)

    n_nodes = x.shape[0]
    n_edges = edge_dst.shape[0]

    # ---- host preprocessing: sort by destination, shard on segment bounds
    order = np.argsort(edge_dst, kind="stable")
    dst_s = edge_dst[order]
    src_s = edge_src[order]
    ea_s = edge_attr[order]

    cuts = [0]
    for c in range(1, N_CORES):
        tgt = c * n_edges // N_CORES
        while tgt < n_edges and dst_s[tgt] == dst_s[tgt - 1]:
            tgt += 1
        cuts.append(min(tgt, n_edges))
    cuts.append(n_edges)
    counts = [cuts[i + 1] - cuts[i] for i in range(N_CORES)]
    e_c = max(2 * E_TILE, int(math.ceil(max(counts) / (2 * E_TILE))) * 2 * E_TILE)

    deg = np.bincount(edge_dst, minlength=n_nodes).astype(np.float32)
    inv_deg = 1.0 / np.maximum(deg, 1.0)

    # ---- shared weight payloads
    w1T = np.ascontiguousarray(W1.T).astype(BF16)                  # [6, 64]
    w2T = np.ascontiguousarray(W2.T).astype(BF16)                  # [64, 128]
    w3T = np.ascontiguousarray(W3.T).astype(BF16)                  # [128, 1024]
    b1v = b1.reshape(H1, 1).astype(np.float32)
    b2v = b2.reshape(H2, 1).astype(np.float32)
    sel = _sel_matrices()
    P = _perm()
    has_b3 = bool(np.any(b3))
    if has_b3:
        r = np.arange(128)
        b3m = np.empty((128, 8), dtype=np.float32)
        for b in range(8):
            b3m[:, b] = b3[(4 * b + r // 32) * F_IN + r % 32]

    in_maps = []
    core_meta = []
    for c in range(N_CORES):
        lo, hi = cuts[c], cuts[c + 1]
        cnt = hi - lo
        dst_c = dst_s[lo:hi]
        xs_c = x[src_s[lo:hi]]                                     # [cnt, 32]

        ea_pad = np.zeros((e_c, EDGE_DIM), dtype=np.float32)
        ea_pad[:cnt] = ea_s[lo:hi]
        xs_pad = np.zeros((e_c, F_IN), dtype=np.float32)
        xs_pad[:cnt] = xs_c
        keep = np.zeros(e_c, dtype=np.float32)
        if cnt > 1:
            keep[1:cnt] = (dst_c[1:] == dst_c[:-1]).astype(np.float32)

        eaT = np.ascontiguousarray(ea_pad.T).astype(BF16)          # [6, e_c]
        xsT = np.ascontiguousarray(xs_pad.T)                       # [32, e_c]
        xsrep = np.tile(xsT, (4, 1)).astype(BF16)                  # [128, e_c]
        if slim:
            # the mask is rank-1 (identical on every useful row): ship 8 rows
            mask = np.ascontiguousarray(
                np.broadcast_to(keep.astype(BF16), (8, e_c))
            )
        else:
            mask = np.zeros((128, e_c), dtype=np.float32)
            mask[P] = keep
            mask = np.ascontiguousarray(mask.astype(BF16))

        # last index of each segment in this shard
        if cnt > 0:
            is_end = np.empty(cnt, dtype=bool)
            is_end[-1] = True
            is_end[:-1] = dst_c[1:] != dst_c[:-1]
            ends = np.flatnonzero(is_end)
            nodes = dst_c[ends]
        else:
            ends = np.zeros(0, dtype=np.int64)
            nodes = np.zeros(0, dtype=np.int64)
        core_meta.append((ends, nodes))

        m = {
            "eaT": eaT,
            "xsrep": xsrep,
            "mask": mask,
            "w1T": w1T,
            "w2T": w2T,
            "w3T": w3T,
            "sel": sel,
            "b1v": b1v,
            "b2v": b2v,
        }
        if has_b3:
            m["b3m"] = b3m
        in_maps.append(m)

    return {
        "in_maps": in_maps,
        "core_meta": core_meta,
        "e_c": e_c,
        "inv_deg": inv_deg,
        "has_b3": has_b3,
        "n_nodes": n_nodes,
        "perm": P,
    }


_CFG = None  # optional module-level cfg override for experiments


def kernel(x, edge_attr, W1, b1, W2, b2, W3, b3, edge_src, edge_dst):
    cfg = {**DEFAULT_CFG, **(_CFG or {})}
    slim = bool(cfg.get("slim_dma"))
    prep = _prepare(x, edge_attr, W1, b1, W2, b2, W3, b3, edge_src, edge_dst, cfg=cfg)
    e_c = prep["e_c"]
    has_b3 = prep["has_b3"]
    key = (e_c, has_b3, tuple(sorted(cfg.items(), key=str)))
    if key not in _program_cache:
        _program_cache[key] = _build_program(e_c, has_b3=has_b3, cfg=cfg)
    nc = _program_cache[key]

    res = run_bass_kernel_spmd(nc, prep["in_maps"], list(range(N_CORES)))

    inv_deg = prep["inv_deg"]
    P = prep["perm"]
    if slim:
        # out rows g*8+r hold partition 32g+r; map msg row o -> out row
        Q = 8 * (P // 32) + P % 32
    out = np.zeros((prep["n_nodes"], F_OUT), dtype=np.float32)
    for c in range(N_CORES):
        scan = np.asarray(res.results[c]["scan_out"]).astype(np.float32)
        scan = scan[Q] if slim else scan[P]                        # [32, e_c]
        ends, nodes = prep["core_meta"][c]
        if len(nodes):
            vals = scan[:, ends].T.copy()                          # [nseg, 32]
            # stitch segments that span scan-reset boundaries: each earlier
            # window's last column holds that window's partial sum
            W = 2 * E_TILE if cfg.get("fuse_scan") else E_TILE
            starts = np.empty_like(ends)
            starts[0] = 0
            starts[1:] = ends[:-1] + 1
            t_end = ends // W
            t_start = starts // W
            for i in np.flatnonzero(t_end > t_start):
                for tp in range(t_start[i], t_end[i]):
                    vals[i] += scan[:, W * tp + W - 1]
            out[nodes] = vals * inv_deg[nodes, None]
    np.maximum(out, 0.0, out=out)
    return out



# revision 27
# speedup vs baseline: 1.0018x; 1.0018x over previous
"""ECC (edge-conditioned convolution) GNN message passing on 8 NeuronCores.

Strategy
--------
Edges are sorted by destination node (host side) and split into 8
contiguous, segment-aligned shards -- one per core.  Each core runs an
identical Bass program over its shard, tiled 512 edges at a time:

  PE  : h1 = relu(W1 @ eaT); h2 = relu(W2 @ h1);
        theta blocks (8x [128,512] per tile) = w3T_block.T @ h2;
        8 col-tiled selector matmuls (M=8, tile_position=(0,32j)) reduce
        prod over i -- four 32-column PE groups run concurrently, so the
        selector pass costs ~2 matmul spans instead of 8.
  ACT : PSUM->SBUF evacuation (relu for h1/h2, f32->bf16 casts for theta)
  DVE : theta * xs_rep elementwise (bf16 2x mode; the deferred "A"-pair
        multiplies of a tile are fused into ONE wide op), one fused
        evac-multiply straight from PSUM (scalar_tensor_tensor), and a
        masked prefix scan (tensor_tensor_scan) with initial=0 PER TILE --
        the host stitches segments that span tile boundaries by adding the
        earlier tiles' last scan columns, which keeps the scans cheap and
        independent (no serial cross-tile carry).
  Pool: one pair of SBUF-only multiplies to offload DVE.

msg rows live scattered across PSUM partitions (p = 32*colgrp + m); the
host knows the permutation.  The scan output [128, e_c] bf16 goes back to
HBM; the host reads each segment's last column (plus boundary partials),
divides by degree and applies the final relu.  Shards are segment aligned
so no cross-core reduction is needed.
"""

import math
import sys

import numpy as np

for _p in ("/opt/trn_rl_repo", "/root/.axon_site/_ro/trn_rl_repo"):
    if _p not in sys.path:
        sys.path.insert(0, _p)

import ml_dtypes

import concourse.bass as bass
import concourse.mybir as mybir
import concourse.tile as tile
from concourse import bacc
from concourse.bass_utils import run_bass_kernel_spmd

N_NODES = 25000
N_EDGES = 250000
F_IN = 32
F_OUT = 32
EDGE_DIM = 6
H1, H2 = 64, 128
N_CORES = 8
E_TILE = 512

BF16 = ml_dtypes.bfloat16

_program_cache: dict = {}

# per-pair conveyor modes for even/odd tiles:
#   "A" = ACT evac + DVE mult, "P" = ACT evac + Pool mult, "S" = DVE fused STT
DEFAULT_CFG = {
    "modes_even": ("A", "A", "P", "S"),
    "modes_odd": ("A", "A", "P", "S"),
    "h1s_even": "act",
    "h1s_odd": "act",
    "h2s_even": "act",
    "h2s_odd": "act",
    # pair processing order on PE (STT pairs first so DVE can drain them
    # while ACT works through the evac queue)
    "pair_order": (1, 3, 2, 0),
    # tiles of lag before a tile's selectors+scan run (deeper = more slack
    # for the slow Pool pair's products)
    "back_delay": 3,
    # merge each tile's deferred A-pair mults into one wide DVE op
    "fuse_amuls": True,
    # one scan op per tile pair (PSUM msg tile spans the pair)
    "fuse_scan": False,
    # 8-row rank-1 mask + 32-row output DMAs (saves ~12MB HBM per core);
    # measured slightly slower (SP op count) -- keep off
    "slim_dma": False,
}


def _sel_matrices():
    sel = np.zeros((128, 16), dtype=np.float32)
    rows = np.arange(128)
    for m in range(4):
        sel[rows[rows // 32 == m], m] = 1.0          # sel_lo: cols 0..3
        sel[rows[rows // 32 == m], 8 + 4 + m] = 1.0  # sel_hi: cols 12..15
    return sel.astype(BF16)


def _perm():
    """P[o] = psum partition holding msg row o."""
    P = np.empty(F_OUT, dtype=np.int64)
    for o in range(F_OUT):
        if o < 16:
            P[o] = 32 * (o // 4) + o % 4
        else:
            P[o] = 32 * (o // 4 - 4) + 4 + o % 4
    return P


def _build_program(
    e_c: int,
    bench_repeat: int | None = None,
    has_b3: bool = False,
    cfg: dict | None = None,
) -> "bass.Bass":
    cfg = {**DEFAULT_CFG, **(cfg or {})}
    f32 = mybir.dt.float32
    bf16 = mybir.dt.bfloat16
    n_tiles = e_c // E_TILE
    assert e_c % (2 * E_TILE) == 0

    nc = bacc.Bacc(None, target_bir_lowering=False)
    slim = bool(cfg.get("slim_dma"))

    ea_t_d = nc.declare_dram_parameter("eaT", [EDGE_DIM, e_c], bf16, isOutput=False)
    xs_d = nc.declare_dram_parameter("xsrep", [128, e_c], bf16, isOutput=False)
    mk_d = nc.declare_dram_parameter("mask", [8 if slim else 128, e_c], bf16, isOutput=False)
    w1_d = nc.declare_dram_parameter("w1T", [EDGE_DIM, H1], bf16, isOutput=False)
    w2_d = nc.declare_dram_parameter("w2T", [H1, H2], bf16, isOutput=False)
    w3_d = nc.declare_dram_parameter("w3T", [H2, F_OUT * F_IN], bf16, isOutput=False)
    sel_d = nc.declare_dram_parameter("sel", [128, 16], bf16, isOutput=False)
    b1_d = nc.declare_dram_parameter("b1v", [H1, 1], f32, isOutput=False)
    b2_d = nc.declare_dram_parameter("b2v", [H2, 1], f32, isOutput=False)
    if has_b3:
        b3_d = nc.declare_dram_parameter("b3m", [128, 8], f32, isOutput=False)
    out_d = nc.declare_dram_parameter("scan_out", [32 if slim else 128, e_c], bf16, isOutput=True)

    relu = mybir.ActivationFunctionType.Relu
    copy = mybir.ActivationFunctionType.Copy
    mult = mybir.AluOpType.mult
    add = mybir.AluOpType.add

    with tile.TileContext(nc) as tc:
        with (
            tc.tile_pool(name="const", bufs=1) as const,
            tc.tile_pool(name="io", bufs=3) as io,
            tc.tile_pool(name="mid", bufs=3) as mid,
            tc.tile_pool(name="scanb", bufs=2) as scanb,
            tc.tile_pool(name="psA", bufs=1, space="PSUM") as psA,
            tc.tile_pool(name="psTH", bufs=2, space="PSUM") as psTH,
            tc.tile_pool(name="psB", bufs=2, space="PSUM") as psB,
        ):
            s_w1 = const.tile([EDGE_DIM, H1], bf16)
            nc.sync.dma_start(out=s_w1, in_=w1_d[:])
            s_w2 = const.tile([H1, H2], bf16)
            nc.sync.dma_start(out=s_w2, in_=w2_d[:])
            s_w3 = const.tile([H2, F_OUT * F_IN], bf16)
            nc.sync.dma_start(out=s_w3, in_=w3_d[:])
            s_sel = const.tile([128, 16], bf16)
            nc.sync.dma_start(out=s_sel, in_=sel_d[:])
            s_b1 = const.tile([H1, 1], f32)
            nc.sync.dma_start(out=s_b1, in_=b1_d[:])
            s_b2 = const.tile([H2, 1], f32)
            nc.sync.dma_start(out=s_b2, in_=b2_d[:])
            if has_b3:
                s_b3 = const.tile([128, 8], f32)
                nc.sync.dma_start(out=s_b3, in_=b3_d[:])

            import contextlib

            loop_cm = (
                tc.For_i(
                    0,
                    bench_repeat,
                    1,
                    hint_engines=(
                        mybir.EngineType.PE,
                        mybir.EngineType.Activation,
                        mybir.EngineType.DVE,
                        mybir.EngineType.SP,
                        mybir.EngineType.Pool,
                    ),
                )
                if bench_repeat is not None
                else contextlib.nullcontext()
            )
            with loop_cm:
                state = {"sc2": None, "pending_out": [], "pending_mults": []}

                def emit_back(pend):
                    """Selectors + scan (+ out DMA) for a completed tile."""
                    t = pend["t"]
                    par = t % 2
                    if cfg.get("fuse_scan"):
                        # selectors of a tile pair share one 2-bank PSUM tile
                        # so the pair needs only ONE scan op
                        if par == 0:
                            state["msgp2"] = psB.tile(
                                [128, 2, E_TILE], f32, tag="msg2", bufs=1, name="msgp2"
                            )
                        msgp = state["msgp2"][:, par, :]
                    else:
                        msgp = psB.tile(
                            [128, E_TILE], f32, tag="msg", name="msgp",
                            bufs=1 if cfg.get("h2_pp") else 2,
                        )
                    for p in (() if cfg.get("ablate_sel") else cfg.get("sel_order", (0, 1, 3, 2))):
                        prod2 = pend["prods"][p]
                        for h in range(2):
                            b = 2 * p + h
                            j = b % 4
                            sel_cols = s_sel[:, 0:8] if b < 4 else s_sel[:, 8:16]
                            nc.tensor.matmul(
                                msgp[32 * j : 32 * j + 8, :],
                                sel_cols,
                                prod2[:, h, :],
                                start=(b < 4),
                                stop=(b >= 4),
                                tile_position=(0, 32 * j),
                                skip_group_check=True,
                            )
                    if cfg.get("ablate_scan"):
                        return
                    if par == 0:
                        state["sc2"] = scanb.tile([128, 2, E_TILE], bf16, tag="sc", name="sc2")
                    sc2 = state["sc2"]
                    # per-tile(-pair) scan with initial=0: the host stitches
                    # segments that span scan-reset boundaries (adds the
                    # previous windows' last columns).  An AP initial would
                    # cost ~+534ns on DVE and serialize the scans.
                    sc_flat = bass.AP(
                        tensor=sc2.tensor,
                        offset=sc2.offset,
                        ap=[list(sc2.ap[0]), [1, 2 * E_TILE]],
                    )
                    if cfg.get("fuse_scan"):
                        if par == 1:
                            msg_flat = bass.AP(
                                tensor=state["msgp2"].tensor,
                                offset=state["msgp2"].offset,
                                ap=[list(state["msgp2"].ap[0]), [1, 2 * E_TILE]],
                            )
                            nc.vector.tensor_tensor_scan(
                                sc_flat, pend["mk2"], msg_flat, initial=0.0,
                                op0=mult, op1=add,
                            )
                    else:
                        sc_slice = sc2[:, par, :]
                        nc.vector.tensor_tensor_scan(
                            sc_slice, pend["mk_t"], msgp, initial=0.0, op0=mult, op1=add
                        )
                    if par == 1:
                        lo2 = (t - 1) * E_TILE
                        if slim:
                            # only the 32 rows holding real msg data (the P
                            # permutation: rows 32g..32g+7 for g in 0..3)
                            for g in range(4):
                                state["pending_out"].append(
                                    (
                                        out_d[8 * g : 8 * g + 8, lo2 : lo2 + 2 * E_TILE],
                                        sc_flat[32 * g : 32 * g + 8, :],
                                    )
                                )
                        else:
                            state["pending_out"].append(
                                (out_d[:, lo2 : lo2 + 2 * E_TILE], sc_flat)
                            )

                def flush_out():
                    # issue out-DMAs late so their waits are already met and
                    # never head-block the SP sequencer's input prefetches
                    for out_ap, in_ap in state["pending_out"]:
                        nc.sync.dma_start(out=out_ap, in_=in_ap)
                    state["pending_out"] = []

                xs2t = mk2t = ea2t = None
                pendq = []
                # uniform fused-group allocation width across parities (the
                # pool tag ring needs one shape); ops use only the live slice
                n_A_max = max(
                    sum(1 for q in cfg["modes_even"] if q == "A"),
                    sum(1 for q in cfg["modes_odd"] if q == "A"),
                )
                for t in range(n_tiles):
                    par = t % 2
                    modes = cfg["modes_even"] if par == 0 else cfg["modes_odd"]
                    h1s_eng = cfg["h1s_even"] if par == 0 else cfg["h1s_odd"]
                    h2s_eng = cfg["h2s_even"] if par == 0 else cfg["h2s_odd"]
                    lo2 = (t - par) * E_TILE
                    if par == 0:
                        xs2t = io.tile([128, 2 * E_TILE], bf16, tag="xs")
                        nc.sync.dma_start(out=xs2t, in_=xs_d[:, lo2 : lo2 + 2 * E_TILE])
                        mk2t = io.tile([128, 2 * E_TILE], bf16, tag="mk")
                        if slim:
                            # rank-1 mask: 8 identical host rows, broadcast to
                            # the 4 row-groups the scan's useful rows live in;
                            # the other 96 rows keep stale data (ignored)
                            for g in range(4):
                                nc.sync.dma_start(
                                    out=mk2t[32 * g : 32 * g + 8, :],
                                    in_=mk_d[:, lo2 : lo2 + 2 * E_TILE],
                                )
                        else:
                            nc.sync.dma_start(out=mk2t, in_=mk_d[:, lo2 : lo2 + 2 * E_TILE])
                        ea2t = io.tile([EDGE_DIM, 2 * E_TILE], bf16, tag="ea")
                        nc.sync.dma_start(
                            out=ea2t, in_=ea_t_d[:, lo2 : lo2 + 2 * E_TILE]
                        )
                        flush_out()

                    c0 = par * E_TILE
                    ea_t = ea2t[:, c0 : c0 + E_TILE]
                    xs_t = xs2t[:, c0 : c0 + E_TILE]
                    mk_t = mk2t[:, c0 : c0 + E_TILE]

                    with tc.high_priority(offset=cfg.get("hprio", 0) or None) if cfg.get("hprio") else contextlib.nullcontext():
                        h1p = psA.tile([H1, E_TILE], f32, tag="h1")
                        nc.tensor.matmul(h1p, s_w1, ea_t, start=True, stop=True)
                        h1s = mid.tile([H1, E_TILE], bf16, tag="h1s")
                        if h1s_eng == "act":
                            nc.scalar.activation(h1s, h1p, relu, bias=s_b1)
                        else:
                            nc.vector.tensor_scalar(
                                h1s, h1p, s_b1, 0.0, add, mybir.AluOpType.max
                            )

                        h2p = psA.tile(
                            [H2, E_TILE], f32, tag="h2",
                            bufs=2 if cfg.get("h2_pp") else 1,
                        )
                        nc.tensor.matmul(h2p, s_w2, h1s, start=True, stop=True)
                        h2s = mid.tile([H2, E_TILE], bf16, tag="h2s")
                        if h2s_eng == "act":
                            nc.scalar.activation(h2s, h2p, relu, bias=s_b2)
                        else:
                            nc.vector.tensor_scalar(
                                h2s, h2p, s_b2, 0.0, add, mybir.AluOpType.max
                            )

                    # lagged tiles' selectors+scan fill the PE while ACT
                    # works through h1s/h2s for this tile
                    if len(pendq) >= cfg["back_delay"]:
                        emit_back(pendq.pop(0))

                    # broadcast view of xs_t: [128, 2(bcast), 512]
                    xs2 = bass.AP(
                        tensor=xs_t.tensor,
                        offset=xs_t.offset,
                        ap=[list(xs_t.ap[0]), [0, 2], list(xs_t.ap[1])],
                    )
                    prods = [None] * 4
                    agrp = None
                    fuse = bool(cfg.get("fuse_amuls"))
                    n_A = sum(1 for q in modes if q == "A")
                    for p in cfg["pair_order"]:
                        thp2 = psTH.tile([128, 2, E_TILE], f32, tag="th")
                        for h in range(2):
                            b = 2 * p + h
                            nc.tensor.matmul(
                                thp2[:, h, :],
                                s_w3[:, b * 128 : (b + 1) * 128],
                                h2s,
                                start=True,
                                stop=True,
                            )
                        mode = modes[p]
                        prod2 = (
                            None
                            if mode == "A"
                            else mid.tile(
                                [128, 2, E_TILE], bf16, tag="prod", bufs=13,
                                name="prod2",
                            )
                        )
                        if mode == "S" and not has_b3:
                            nc.vector.scalar_tensor_tensor(
                                prod2, thp2, 1.0, xs2, mult, mult
                            )
                        elif mode == "S":  # has_b3: per-block, bias added
                            for h in range(2):
                                b = 2 * p + h
                                nc.vector.scalar_tensor_tensor(
                                    prod2[:, h, :],
                                    thp2[:, h, :],
                                    s_b3[:, b : b + 1],
                                    xs_t,
                                    add,
                                    mult,
                                )
                        else:
                            if mode == "A":
                                # defer the DVE mult one tile: keeps the DVE
                                # FIFO free of ACT-gated ops (prods are only
                                # consumed back_delay tiles later).  With
                                # fuse_amuls all A pairs of a tile share one
                                # group tile so the deferred mult is a single
                                # wide DVE op.
                                if agrp is None:
                                    w = 2 * n_A if fuse else 2
                                    w_alloc = 2 * n_A_max if fuse else 2
                                    agrp = {
                                        "ths": mid.tile(
                                            [128, w_alloc, E_TILE], bf16, tag="thsg",
                                            bufs=6 if fuse else 12, name="thsg",
                                        ),
                                        "w": w,
                                        "w_alloc": w_alloc,
                                        "used": 0,
                                        "slots": [],
                                        "prods": prods,
                                        "xs_t": xs_t,
                                    }
                                    state["pending_mults"].append(agrp)
                                idx = agrp["used"]
                                agrp["used"] += 1
                                ths2 = agrp["ths"][:, 2 * idx : 2 * idx + 2, :]
                                agrp["slots"].append((p, idx))
                                if not fuse:
                                    agrp = None
                            else:
                                ths2 = mid.tile([128, 2, E_TILE], bf16, tag="ths", bufs=6)
                            if not has_b3:
                                nc.scalar.activation(ths2, thp2, copy)
                            else:
                                for h in range(2):
                                    b = 2 * p + h
                                    nc.scalar.activation(
                                        ths2[:, h, :],
                                        thp2[:, h, :],
                                        copy,
                                        bias=s_b3[:, b : b + 1],
                                    )
                            if mode == "A":
                                prod2 = None
                            else:  # "P": Pool, per-block plain APs
                                for h in range(2):
                                    nc.gpsimd.tensor_tensor(
                                        prod2[:, h, :], ths2[:, h, :], xs_t, mult
                                    )
                        prods[p] = prod2

                    # flush previous tile's deferred DVE mults (their evacs
                    # finished long ago, so they never stall the DVE FIFO)
                    def flush_mults(m):
                        w = m["w"]
                        mp = mid.tile(
                            [128, m["w_alloc"], E_TILE], bf16, tag="prodg",
                            bufs=6 if fuse else 12, name="prodd",
                        )
                        xt = m["xs_t"]
                        xsb = bass.AP(
                            tensor=xt.tensor,
                            offset=xt.offset,
                            ap=[list(xt.ap[0]), [0, w], list(xt.ap[1])],
                        )
                        nc.vector.tensor_mul(mp[:, 0:w, :], m["ths"][:, 0:w, :], xsb)
                        for p_, idx in m["slots"]:
                            m["prods"][p_] = mp[:, 2 * idx : 2 * idx + 2, :]

                    newly = [m for m in state["pending_mults"] if m["prods"] is not prods]
                    for m in newly:
                        flush_mults(m)
                    state["pending_mults"] = [
                        m for m in state["pending_mults"] if m["prods"] is prods
                    ]

                    pendq.append({"t": t, "prods": prods, "mk_t": mk_t, "mk2": mk2t})
                for m in state["pending_mults"]:
                    flush_mults(m)
                state["pending_mults"] = []
                for pd in pendq:
                    emit_back(pd)
                flush_out()

    nc.finalize()
    return nc


def _prepare(x, edge_attr, W1, b1, W2, b2, W3, b3, edge_src, edge_dst, cfg=None):
    cfg = {**DEFAULT_CFG, **(cfg or {})}
    slim = bool(cfg.get("slim_dma"))
    x = np.asarray(x, dtype=np.float32)
    edge_attr = np.asarray(edge_attr, dtype=np.float32)
    W1 = np.asarray(W1, dtype=np.float32)
    b1 = np.asarray(b1, dtype=np.float32)
    W2 = np.asarray(W2, dtype=np.float32)
    b2 = np.asarray(b2, dtype=np.float32)
    W3 = np.asarray(W3, dtype=np.float32)
    b3 = np.asarray(b3, dtype=np.float32)
    edge_src = np.asarray(edge_src).astype(np.int64)
    edge_dst = np.asarray(edge_dst).astype(np.int64)

    n_nodes = x.shape[0]
    n_edges = edge_dst.shape[0]

    # ---- host preprocessing: sort by destination, shard on segment bounds
    order = np.argsort(edge_dst, kind="stable")
    dst_s = edge_dst[order]
    src_s = edge_src[order]
    ea_s = edge_attr[order]

    cuts = [0]
    for c in range(1, N_CORES):
        tgt = c * n_edges // N_CORES
        while tgt < n_edges and dst_s[tgt] == dst_s[tgt - 1]:
            tgt += 1
        cuts.append(min(tgt, n_edges))
    cuts.append(n_edges)
    counts = [cuts[i + 1] - cuts[i] for i in range(N_CORES)]
    e_c = max(2 * E_TILE, int(math.ceil(max(counts) / (2 * E_TILE))) * 2 * E_TILE)

    deg = np.bincount(edge_dst, minlength=n_nodes).astype(np.float32)
    inv_deg = 1.0 / np.maximum(deg, 1.0)

    # ---- shared weight payloads
    w1T = np.ascontiguousarray(W1.T).astype(BF16)                  # [6, 64]
    w2T = np.ascontiguousarray(W2.T).astype(BF16)                  # [64, 128]
    w3T = np.ascontiguousarray(W3.T).astype(BF16)                  # [128, 1024]
    b1v = b1.reshape(H1, 1).astype(np.float32)
    b2v = b2.reshape(H2, 1).astype(np.float32)
    sel = _sel_matrices()
    P = _perm()
    has_b3 = bool(np.any(b3))
    if has_b3:
        r = np.arange(128)
        b3m = np.empty((128, 8), dtype=np.float32)
        for b in range(8):
            b3m[:, b] = b3[(4 * b + r // 32) * F_IN + r % 32]

    in_maps = []
    core_meta = []
    for c in range(N_CORES):
        lo, hi = cuts[c], cuts[c + 1]
        cnt = hi - lo
        dst_c = dst_s[lo:hi]
        xs_c = x[src_s[lo:hi]]                                     # [cnt, 32]

        ea_pad = np.zeros((e_c, EDGE_DIM), dtype=np.float32)
        ea_pad[:cnt] = ea_s[lo:hi]
        xs_pad = np.zeros((e_c, F_IN), dtype=np.float32)
        xs_pad[:cnt] = xs_c
        keep = np.zeros(e_c, dtype=np.float32)
        if cnt > 1:
            keep[1:cnt] = (dst_c[1:] == dst_c[:-1]).astype(np.float32)

        eaT = np.ascontiguousarray(ea_pad.T).astype(BF16)          # [6, e_c]
        xsT = np.ascontiguousarray(xs_pad.T)                       # [32, e_c]
        xsrep = np.tile(xsT, (4, 1)).astype(BF16)                  # [128, e_c]
        if slim:
            # the mask is rank-1 (identical on every useful row): ship 8 rows
            mask = np.ascontiguousarray(
                np.broadcast_to(keep.astype(BF16), (8, e_c))
            )
        else:
            mask = np.zeros((128, e_c), dtype=np.float32)
            mask[P] = keep
            mask = np.ascontiguousarray(mask.astype(BF16))

        # last index of each segment in this shard
        if cnt > 0:
            is_end = np.empty(cnt, dtype=bool)
            is_end[-1] = True
            is_end[:-1] = dst_c[1:] != dst_c[:-1]
            ends = np.flatnonzero(is_end)
            nodes = dst_c[ends]
        else:
            ends = np.zeros(0, dtype=np.int64)
            nodes = np.zeros(0, dtype=np.int64)
        core_meta.append((ends, nodes))

        m = {
            "eaT": eaT,
            "xsrep": xsrep,
            "mask": mask,
            "w1T": w1T,
            "w2T": w2T,
            "w3T": w3T,
            "sel": sel,
            "b1v": b1v,
            "b2v": b2v,
        }
        if has_b3:
            m["b3m"] = b3m
        in_maps.append(m)

    return {
        "in_maps": in_maps,
        "core_meta": core_meta,
        "e_c": e_c,
        "inv_deg": inv_deg,
        "has_b3": has_b3,
        "n_nodes": n_nodes,
        "perm": P,
    }


_CFG = None  # optional module-level cfg override for experiments


def kernel(x, edge_attr, W1, b1, W2, b2, W3, b3, edge_src, edge_dst):
    cfg = {**DEFAULT_CFG, **(_CFG or {})}
    slim = bool(cfg.get("slim_dma"))
    prep = _prepare(x, edge_attr, W1, b1, W2, b2, W3, b3, edge_src, edge_dst, cfg=cfg)
    e_c = prep["e_c"]
    has_b3 = prep["has_b3"]
    key = (e_c, has_b3, tuple(sorted(cfg.items(), key=str)))
    if key not in _program_cache:
        _program_cache[key] = _build_program(e_c, has_b3=has_b3, cfg=cfg)
    nc = _program_cache[key]

    res = run_bass_kernel_spmd(nc, prep["in_maps"], list(range(N_CORES)))

    inv_deg = prep["inv_deg"]
    P = prep["perm"]
    if slim:
        # out rows g*8+r hold partition 32g+r; map msg row o -> out row
        Q = 8 * (P // 32) + P % 32
    out = np.zeros((prep["n_nodes"], F_OUT), dtype=np.float32)
    for c in range(N_CORES):
        scan = np.asarray(res.results[c]["scan_out"]).astype(np.float32)
        scan = scan[Q] if slim else scan[P]                        # [32, e_c]
        ends, nodes = prep["core_meta"][c]
        if len(nodes):
            vals = scan[:, ends].T.copy()                          # [nseg, 32]
            # stitch segments that span scan-reset boundaries: each earlier
            # window's last column holds that window's partial sum
            W = 2 * E_TILE if cfg.get("fuse_scan") else E_TILE
            starts = np.empty_like(ends)
            starts[0] = 0
            starts[1:] = ends[:-1] + 1
            t_end = ends // W
            t_start = starts // W
            for i in np.flatnonzero(t_end > t_start):
                for tp in range(t_start[i], t_end[i]):
                    vals[i] += scan[:, W * tp + W - 1]
            out[nodes] = vals * inv_deg[nodes, None]
    np.maximum(out, 0.0, out=out)
    return out

